# revision 14
# baseline (speedup 1.0000x reference)
"""AffinityNet Trainium2 kernel: 8-core data-parallel (1 batch per core).

Per-core pipeline (batch b):
  f2 = elu(w2 @ f2_in[b])        [64, 64, 64]   (1x1 conv as matmul, bf16)
  f2r = bilinear_resize(f2)      [64, 32, 32]   (4-tap separable, DVE)
  f3 = elu(w3 @ f3_in[b])        [128, 32, 32]
  f4 = elu(w4 @ f4_in[b])        [320, 32, 32]
  x  = elu(w9p @ xcat)           [512, 32, 32]  (w9 host-permuted to the
                                                 on-chip channel layout)
  aff[p, i] = exp(-mean_c |x[c, to(p,i)] - x[c, from(i)]|)   [34, 672]

The neighbor gather is a pure 2D shift: done with overlapping strided AP
views on the vector engine (offsets grouped by (dy, dx-parity) into single
instructions); |.| on the scalar engine; channel sums via ones-matmul on PE
accumulating into per-slot PSUM rows at partitions {0,32,64,96}; the final
exp(-s/512) is fused into the PSUM->SBUF extraction on ACT. Host only
shards/reassembles and un-scrambles the slot layout.
"""

import sys

for _p in ("/opt/trn_rl_repo", "/root/.axon_site", "/root/.axon_site/_ro/pypackages"):
    if _p not in sys.path:
        sys.path.append(_p)

import numpy as np
import ml_dtypes

BF16 = ml_dtypes.bfloat16

# ---------------- problem constants (hardcoded from the spec) ----------------
B = 8
N_CORES = 8
CH = 28             # cropped height  (h range of "from" positions)
CW = 24             # cropped width   (w range of "from" positions)
NPOS = CH * CW      # 672

# offset list in the reference order (dy, dx)
OFFSETS = [(0, x) for x in range(1, 5)] + [
    (y, x) for y in range(1, 5) for x in range(-4, 5) if x * x + y * y < 25
]
assert len(OFFSETS) == 34

# groups: same dy, same-parity dx run (step 2) -> one DVE sub per (group, ctile)
GROUPS = [
    (0, [2, 4]),
    (0, [1, 3]),
    (1, [-4, -2, 0, 2, 4]),
    (1, [-3, -1, 1, 3]),
    (2, [-4, -2, 0, 2, 4]),
    (2, [-3, -1, 1, 3]),
    (3, [-2, 0, 2]),
    (3, [-3, -1, 1, 3]),
    (4, [-2, 0, 2]),
    (4, [-1, 1]),
]
assert sorted(set((dy, dx) for dy, dxs in GROUPS for dx in dxs)) == sorted(OFFSETS)

CHUNK = 336  # divides every group size evenly: 68 slots = 17 full banks

# slot list: (group, start col within group, size)
_SLOTS = []
for _g, (_dy, _dxs) in enumerate(GROUPS):
    _G = len(_dxs) * NPOS
    _c0 = 0
    while _c0 < _G:
        _cs = min(CHUNK, _G - _c0)
        _SLOTS.append((_g, _c0, _cs))
        _c0 += _cs
N_SLOTS = len(_SLOTS)                 # 68
N_BANKSEQ = N_SLOTS // 4              # 17
assert N_SLOTS % 4 == 0


def _build_unscramble():
    """aff[p, i] = out_flat[slot*CHUNK + col]; build (slot, col) index maps."""
    off_index = {od: i for i, od in enumerate(OFFSETS)}
    slot_of = np.zeros((34, NPOS), np.int64)
    col_of = np.zeros((34, NPOS), np.int64)
    slot_base = 0
    for g, (dy, dxs) in enumerate(GROUPS):
        G = len(dxs) * NPOS
        nchunks = (G + CHUNK - 1) // CHUNK
        for k, dx in enumerate(dxs):
            p = off_index[(dy, dx)]
            gcols = k * NPOS + np.arange(NPOS)
            slot_of[p, :] = slot_base + gcols // CHUNK
            col_of[p, :] = gcols % CHUNK
        slot_base += nchunks
    assert slot_base == N_SLOTS
    return slot_of, col_of


_SLOT_OF, _COL_OF = _build_unscramble()
_FLAT_IDX = _SLOT_OF * CHUNK + _COL_OF

# bilinear 64->32 resize taps (jax.image.resize, triangle kernel, antialias):
# interior: out[j] = .125 in[2j-1] + .375 in[2j] + .375 in[2j+1] + .125 in[2j+2]
# out[0] = (3 in[0] + 3 in[1] + in[2]) / 7 ; out[31] = (in[61] + 3 in[62] + 3 in[63]) / 7
W_IN = 0.375
W_OUT = 0.125
B3 = 3.0 / 7.0
B1 = 1.0 / 7.0

# xcat channel layout = [f3(128) | f4 m0(128) | f4 m1(128) | f2r(64)+f4 m2(64)];
# original concat order = [f2r 0:64 | f3 64:192 | f4 192:512]
XCAT_PERM = (
    list(range(64, 192))
    + list(range(192, 320))
    + list(range(320, 448))
    + list(range(0, 64))
    + list(range(448, 512))
)


# ------------------------------- bass kernel ---------------------------------

def _fview(t_ap, off, dims):
    """Strided free-dim view of a tile AP (partition dim preserved)."""
    import concourse.bass as bass

    pd = list(t_ap.ap)[0]
    return bass.AP(
        tensor=t_ap.tensor,
        offset=t_ap.offset + off,
        ap=[list(pd)] + [list(d) for d in dims],
    )


def _pstride_view(t_ap, nrows, ncols):
    """Partition-strided view: rows {0,32,64,...} of a [128, X] tile."""
    import concourse.bass as bass

    pd = list(t_ap.ap)[0]
    row_stride = pd[0]
    return bass.AP(
        tensor=t_ap.tensor,
        offset=t_ap.offset,
        ap=[[row_stride * 32, nrows], [1, ncols]],
    )


def build_nc():
    """Build + compile the per-core Bass program."""
    from contextlib import ExitStack

    import concourse.tile as tile
    from concourse import bacc, mybir

    dt = mybir.dt
    ALU = mybir.AluOpType
    ACTF = mybir.ActivationFunctionType

    nc = bacc.Bacc("TRN2", target_bir_lowering=False, debug=False)

    f2d = nc.dram_tensor("f2", [128, 4 * 4096], dt.bfloat16, kind="ExternalInput").ap()
    f3d = nc.dram_tensor("f3", [128, 8 * 1024], dt.bfloat16, kind="ExternalInput").ap()
    f4d = nc.dram_tensor("f4", [128, 16 * 1024], dt.bfloat16, kind="ExternalInput").ap()
    w2d = nc.dram_tensor("w2t", [128, 4 * 64], dt.bfloat16, kind="ExternalInput").ap()
    w3d = nc.dram_tensor("w3t", [128, 8 * 128], dt.bfloat16, kind="ExternalInput").ap()
    w4d = nc.dram_tensor("w4t", [128, 16 * 320], dt.bfloat16, kind="ExternalInput").ap()
    w9d = nc.dram_tensor("w9t", [128, 4 * 512], dt.bfloat16, kind="ExternalInput").ap()
    outd = nc.dram_tensor(
        "aff", [N_SLOTS, CHUNK], dt.float32, kind="ExternalOutput"
    ).ap()

    with tile.TileContext(nc) as tc, ExitStack() as ctx:
        wpool = ctx.enter_context(tc.tile_pool(name="w", bufs=1))
        fpool = ctx.enter_context(tc.tile_pool(name="fin", bufs=10))
        xpool = ctx.enter_context(tc.tile_pool(name="x", bufs=1))
        pspool = ctx.enter_context(tc.tile_pool(name="ps", bufs=6, space="PSUM"))
        tpool = ctx.enter_context(tc.tile_pool(name="tmp", bufs=4))
        dpool = ctx.enter_context(tc.tile_pool(name="diff", bufs=7))
        opool = ctx.enter_context(tc.tile_pool(name="out", bufs=4))

        # ---- constants; weights go on the scalar DMA queue, activations on
        # the sync queue. f3 first (smallest) so conv3 starts ASAP; f2 last
        # (conv2 runs after conv4 from resident SBUF).
        ones = wpool.tile([128, 32], dt.bfloat16, tag="ones")
        nc.vector.memset(ones[:], 1.0)
        scorr = wpool.tile([128, 1024], dt.bfloat16, tag="scorr")
        nc.gpsimd.memset(scorr[:], 0.0)

        w3sb = wpool.tile([128, 8 * 128], dt.bfloat16, tag="w3")
        nc.scalar.dma_start(w3sb[:], w3d[:])
        w4sb = wpool.tile([128, 16 * 320], dt.bfloat16, tag="w4")
        nc.scalar.dma_start(w4sb[:], w4d[:])
        w2sb = wpool.tile([128, 4 * 64], dt.bfloat16, tag="w2")
        nc.scalar.dma_start(w2sb[:], w2d[:])
        w9sb = wpool.tile([128, 4 * 512], dt.bfloat16, tag="w9")
        nc.scalar.dma_start(w9sb[:], w9d[:])

        f3c = [fpool.tile([128, 4096], dt.bfloat16, tag="fc", name=f"f3c{h}") for h in range(2)]
        for h in range(2):
            nc.sync.dma_start(f3c[h][:], f3d[:, h * 4096 : (h + 1) * 4096])
        f4c = [fpool.tile([128, 4096], dt.bfloat16, tag="fc", name=f"f4c{h}") for h in range(4)]
        for h in range(4):
            nc.sync.dma_start(f4c[h][:], f4d[:, h * 4096 : (h + 1) * 4096])
        f2c = [fpool.tile([128, 4096], dt.bfloat16, tag="fc", name=f"f2c{h}") for h in range(4)]
        for h in range(4):
            nc.sync.dma_start(f2c[h][:], f2d[:, h * 4096 : (h + 1) * 4096])

        xcat = xpool.tile([128, 4 * 1024], dt.bfloat16, tag="xcat")
        xbuf = xpool.tile([128, 4 * 1024], dt.bfloat16, tag="xbuf")
        xodd = xpool.tile([128, 4 * 1024], dt.bfloat16, tag="xodd")

        def elu(ps_t, pb, psz, dst_ap):
            """dst = elu(psum) = max(x, exp(min(x,0)) - 1), on rows pb:pb+psz."""
            ps_ap = ps_t[pb : pb + psz, :]
            u = tpool.tile([128, 512], dt.bfloat16, tag="eu", name="eu")
            e = tpool.tile([128, 512], dt.bfloat16, tag="ee", name="ee")
            r = tpool.tile([128, 512], dt.bfloat16, tag="er", name="er")
            nc.scalar.activation(u[pb : pb + psz, :], ps_ap, ACTF.Relu, scale=-1.0)
            nc.scalar.activation(
                e[pb : pb + psz, :], u[pb : pb + psz, :], ACTF.Exp, scale=-1.0
            )
            nc.scalar.activation(r[pb : pb + psz, :], ps_ap, ACTF.Relu)
            nc.vector.scalar_tensor_tensor(
                dst_ap, e[pb : pb + psz, :], -1.0, r[pb : pb + psz, :], ALU.add, ALU.add
            )

        # ================= conv3 -> xcat[:, 0:1024] (ct0) =====================
        for n in range(2):
            ps3 = pspool.tile([128, 512], dt.float32, tag="ps", name=f"ps3_{n}")
            for k in range(8):
                nc.tensor.matmul(
                    ps3[:, :],
                    w3sb[:, k * 128 : (k + 1) * 128],
                    f3c[k // 4][:, (k % 4) * 1024 + n * 512 : (k % 4) * 1024 + (n + 1) * 512],
                    start=(k == 0),
                    stop=(k == 7),
                )
            elu(ps3, 0, 128, xcat[:, n * 512 : (n + 1) * 512])

        # ===== conv4 -> xcat ct1, ct2, ct3-rows-64:128 ========================
        MCH4 = [(0, 128, 0), (128, 128, 0), (256, 64, 64)]  # (moff, msz, pbase)
        ps4 = [pspool.tile([128, 512], dt.float32, tag="ps", name=f"ps4_{i}") for i in range(6)]
        for k in range(16):
            for mi, (moff, msz, pb) in enumerate(MCH4):
                for n in range(2):
                    nc.tensor.matmul(
                        ps4[mi * 2 + n][pb : pb + msz, :],
                        w4sb[:, k * 320 + moff : k * 320 + moff + msz],
                        f4c[k // 4][:, (k % 4) * 1024 + n * 512 : (k % 4) * 1024 + (n + 1) * 512],
                        start=(k == 0),
                        stop=(k == 15),
                        tile_position=(0, pb),
                    )
        for mi, (moff, msz, pb) in enumerate(MCH4):
            for n in range(2):
                dst = xcat[pb : pb + msz, (1 + mi) * 1024 + n * 512 : (1 + mi) * 1024 + (n + 1) * 512]
                elu(ps4[mi * 2 + n], pb, msz, dst)

        # ========== conv2 (2-way col-tiled, M=64): x2 = elu(w2 @ f2) ==========
        # x2 split layout [128, 2048]: partitions 64*w_+c hold image half
        # h in [32*w_, 32*w_+32), flat col = 64*(h%32) + w.
        x2 = xpool.tile([128, 2048], dt.bfloat16, tag="x2")
        ps2 = [pspool.tile([128, 512], dt.float32, tag="ps", name=f"ps2_{j}") for j in range(4)]
        for k in range(4):
            for j in range(4):
                for w_ in range(2):
                    n = w_ * 4 + j
                    nc.tensor.matmul(
                        ps2[j][64 * w_ : 64 * w_ + 64, :],
                        w2sb[:, k * 64 : (k + 1) * 64],
                        f2c[k][:, n * 512 : (n + 1) * 512],
                        start=(k == 0),
                        stop=(k == 3),
                        tile_position=(0, 64 * w_),
                    )
        for j in range(4):
            for w_ in range(2):
                elu(ps2[j], 64 * w_, 64, x2[64 * w_ : 64 * w_ + 64, 512 * j : 512 * (j + 1)])

        # ================= resize: x2 -> xcat[0:64, 3072:4096] ================
        # Pairwise-sum form, unnormalized by 8x per pass (64x total, divided
        # out of w9's f2r columns on the host):
        #   W': A[j]=in[2j]+in[2j+1], B[j]=in[2j-1]+in[2j+2], rw2=3A+B
        #   H': C[i]=r[2i]+r[2i+1],  D[i]=r[2i-1]+r[2i+2],  out=3C+D
        # Boundary taps (j,i in {0,31}): out=(24*pair + 8*third)/7.
        # DVE ops need src/dst at the same base partition, so the W' result
        # stays in the split layout rw [128, 32hh x 32j]; a DMA folds the
        # upper image half down to rwup [64, 1024] for the H pass.
        F2R0 = 3072
        xc64 = xcat[0:64, :]
        x2a = x2[:]
        rA = xpool.tile([128, 1024], dt.bfloat16, tag="rA")
        rB = xpool.tile([128, 1024], dt.bfloat16, tag="rB")
        rw = xpool.tile([128, 1024], dt.bfloat16, tag="rw")
        rwup = xpool.tile([64, 1024], dt.bfloat16, tag="rwup")
        nc.vector.tensor_tensor(
            _fview(rA[:], 0, [(32, 32), (1, 32)]),
            _fview(x2a, 0, [(64, 32), (2, 32)]),
            _fview(x2a, 1, [(64, 32), (2, 32)]),
            ALU.add,
        )
        nc.vector.tensor_tensor(
            _fview(rB[:], 1, [(32, 32), (1, 30)]),
            _fview(x2a, 1, [(64, 32), (2, 30)]),
            _fview(x2a, 4, [(64, 32), (2, 30)]),
            ALU.add,
        )
        rwa = rw[:]
        nc.vector.scalar_tensor_tensor(
            _fview(rwa, 1, [(32, 32), (1, 30)]),
            _fview(rA[:], 1, [(32, 32), (1, 30)]), 3.0,
            _fview(rB[:], 1, [(32, 32), (1, 30)]), ALU.mult, ALU.add,
        )
        rj0 = _fview(rwa, 0, [(32, 32), (1, 1)])
        nc.vector.tensor_scalar_mul(rj0, _fview(rA[:], 0, [(32, 32), (1, 1)]), 24.0 / 7.0)
        nc.vector.scalar_tensor_tensor(
            rj0, _fview(x2a, 2, [(64, 32), (1, 1)]), 8.0 / 7.0, rj0, ALU.mult, ALU.add
        )
        rj31 = _fview(rwa, 31, [(32, 32), (1, 1)])
        nc.vector.tensor_scalar_mul(rj31, _fview(rA[:], 31, [(32, 32), (1, 1)]), 24.0 / 7.0)
        nc.vector.scalar_tensor_tensor(
            rj31, _fview(x2a, 61, [(64, 32), (1, 1)]), 8.0 / 7.0, rj31, ALU.mult, ALU.add
        )
        nc.sync.dma_start(rwup[:], rw[64:128, :])
        rlo = rw[0:64, :]
        rup = rwup[:]
        rC = xpool.tile([64, 1024], dt.bfloat16, tag="rC")
        rD = xpool.tile([64, 1024], dt.bfloat16, tag="rD")
        # C[i] = r[2i] + r[2i+1]: i 0..15 from rlo, 16..31 from rup
        nc.vector.tensor_tensor(
            _fview(rC[:], 0, [(32, 16), (1, 32)]),
            _fview(rlo, 0, [(64, 16), (1, 32)]),
            _fview(rlo, 32, [(64, 16), (1, 32)]), ALU.add,
        )
        nc.vector.tensor_tensor(
            _fview(rC[:], 512, [(32, 16), (1, 32)]),
            _fview(rup, 0, [(64, 16), (1, 32)]),
            _fview(rup, 32, [(64, 16), (1, 32)]), ALU.add,
        )
        # D[i] = r[2i-1] + r[2i+2]: i 1..14 lo, 15/16 cross, 17..30 up
        nc.vector.tensor_tensor(
            _fview(rD[:], 32, [(32, 14), (1, 32)]),
            _fview(rlo, 32, [(64, 14), (1, 32)]),
            _fview(rlo, 128, [(64, 14), (1, 32)]), ALU.add,
        )
        nc.vector.tensor_tensor(
            _fview(rD[:], 15 * 32, [(32, 1), (1, 32)]),
            _fview(rlo, 29 * 32, [(32, 1), (1, 32)]),
            _fview(rup, 0, [(32, 1), (1, 32)]), ALU.add,
        )
        nc.vector.tensor_tensor(
            _fview(rD[:], 16 * 32, [(32, 1), (1, 32)]),
            _fview(rlo, 31 * 32, [(32, 1), (1, 32)]),
            _fview(rup, 2 * 32, [(32, 1), (1, 32)]), ALU.add,
        )
        nc.vector.tensor_tensor(
            _fview(rD[:], 17 * 32, [(32, 14), (1, 32)]),
            _fview(rup, 32, [(64, 14), (1, 32)]),
            _fview(rup, 128, [(64, 14), (1, 32)]), ALU.add,
        )
        nc.vector.scalar_tensor_tensor(
            _fview(xc64, F2R0 + 32, [(32, 30), (1, 32)]),
            _fview(rC[:], 32, [(32, 30), (1, 32)]), 3.0,
            _fview(rD[:], 32, [(32, 30), (1, 32)]), ALU.mult, ALU.add,
        )
        oi0 = _fview(xc64, F2R0, [(32, 1), (1, 32)])
        nc.vector.tensor_scalar_mul(oi0, _fview(rC[:], 0, [(32, 1), (1, 32)]), 24.0 / 7.0)
        nc.vector.scalar_tensor_tensor(
            oi0, _fview(rlo, 2 * 32, [(32, 1), (1, 32)]), 8.0 / 7.0, oi0, ALU.mult, ALU.add
        )
        oi31 = _fview(xc64, F2R0 + 31 * 32, [(32, 1), (1, 32)])
        nc.vector.tensor_scalar_mul(oi31, _fview(rC[:], 31 * 32, [(32, 1), (1, 32)]), 24.0 / 7.0)
        nc.vector.scalar_tensor_tensor(
            oi31, _fview(rup, 29 * 32, [(32, 1), (1, 32)]), 8.0 / 7.0, oi31, ALU.mult, ALU.add
        )

        # ================= conv9 -> xbuf ======================================
        for m in range(4):
            for n in range(2):
                ps9 = pspool.tile([128, 512], dt.float32, tag="ps", name=f"ps9_{m}_{n}")
                for k in range(4):
                    nc.tensor.matmul(
                        ps9[:, :],
                        w9sb[:, k * 512 + m * 128 : k * 512 + (m + 1) * 128],
                        xcat[:, k * 1024 + n * 512 : k * 1024 + (n + 1) * 512],
                        start=(k == 0),
                        stop=(k == 3),
                    )
                elu(ps9, 0, 128, xbuf[:, m * 1024 + n * 512 : m * 1024 + (n + 1) * 512])

        # xodd[p, j] = xbuf[p, j+1]  (for 4B-aligned odd-dx views); per-ctile
        # DMA shift-copies keep it off the DVE and start as soon as each
        # conv9 m-tile lands.
        for ct in range(4):
            nc.sync.dma_start(
                xodd[:, ct * 1024 : ct * 1024 + 1023],
                xbuf[:, ct * 1024 + 1 : ct * 1024 + 1024],
            )

        # S[pos] = sum_c x[c, pos] (rows 0-31 identical); s_corr = -S/2 in bf16
        ps_s = pspool.tile([128, 1024], dt.float32, tag="ps_s", name="ps_s", bufs=1)
        for ct in range(4):
            for n in range(2):
                nc.tensor.matmul(
                    ps_s[0:32, n * 512 : (n + 1) * 512],
                    ones[:, 0:32],
                    xbuf[:, ct * 1024 + n * 512 : ct * 1024 + (n + 1) * 512],
                    start=(ct == 0),
                    stop=(ct == 3),
                )
        # s_corr rows: hi = bf16(-S/2), lo = residual (-S/2 - hi); k=2 matmul
        # sums both, recovering ~fp32 accuracy from bf16 operands.
        s_tmp = xpool.tile([1, 2048], dt.bfloat16, tag="s_tmp")
        nc.scalar.activation(s_tmp[0:1, 0:1024], ps_s[0:1, :], ACTF.Copy, scale=-0.5)
        nc.vector.scalar_tensor_tensor(
            s_tmp[0:1, 1024:2048], ps_s[0:1, :], -0.5, s_tmp[0:1, 0:1024],
            ALU.mult, ALU.subtract,
        )
        # scorr rows 0/1: hi = bf16(-S/2), lo = residual; rows 2..127 zero so
        # the correction matmuls can be K=128 (same (128,32) PE tile mode and
        # same all-ones lhsT as the channel-sum matmuls -> no mode switches).
        nc.sync.dma_start(scorr[0:1, :], s_tmp[0:1, 0:1024])
        nc.sync.dma_start(scorr[1:2, :], s_tmp[0:1, 1024:2048])

        # ================= affinity ==========================================
        # Per bank-quad: 4 slots run on the 4 (128,32) PE column tiles
        # concurrently (interleaved issue), 6 chained K=128 matmuls each
        # (4 channel-sum + 2 corrections vs zero-padded scorr). One ACT exp
        # over the partition-strided rows {0,32,64,96} extracts the quad.
        xba = xbuf[:]
        atiles_by_group = {}

        def ensure_group(g):
            if g in atiles_by_group:
                return
            dy, dxs = GROUPS[g]
            ndx = len(dxs)
            G = ndx * NPOS
            odd = dxs[0] % 2 != 0
            tiles = []
            for ct in range(4):
                ctb = ct * 1024
                ff = _fview(xba, ctb + 4, [(0, ndx), (32, CH), (1, CW)])
                if odd:
                    ft = _fview(
                        xodd[:], ctb + 32 * dy + 4 + dxs[0] - 1,
                        [(2, ndx), (32, CH), (1, CW)],
                    )
                else:
                    ft = _fview(
                        xba, ctb + 32 * dy + 4 + dxs[0],
                        [(2, ndx), (32, CH), (1, CW)],
                    )
                mtile = dpool.tile([128, G], dt.bfloat16, tag="d", name=f"m_{g}_{ct}")
                mv = _fview(mtile[:], 0, [(NPOS, ndx), (CW, CH), (1, CW)])
                nc.vector.tensor_tensor(mv, ft, ff, ALU.max)
                tiles.append(mtile)
            atiles_by_group[g] = tiles

        for t in range(N_BANKSEQ):
            slots = list(range(4 * t, 4 * t + 4))
            for s in slots:
                ensure_group(_SLOTS[s][0])
            pst = pspool.tile([128, 512], dt.float32, tag="ps", name=f"pq_{t}")
            for k in range(6):
                for q, s in enumerate(slots):
                    g, c0, cs = _SLOTS[s]
                    dy, dxs = GROUPS[g]
                    if k < 4:
                        rhs = atiles_by_group[g][k][:, c0 : c0 + cs]
                    else:
                        dxi, h2 = (c0 // NPOS), (c0 % NPOS) // 336
                        off = 4 + 32 * 14 * h2
                        if k == 5:
                            off += 32 * dy + dxs[dxi]
                        rhs = _fview(scorr[:], off, [(32, 14), (1, CW)])
                    nc.tensor.matmul(
                        pst[32 * q : 32 * q + 32, 0:cs],
                        ones[:, 0:32],
                        rhs,
                        start=(k == 0),
                        stop=(k == 5),
                        tile_position=(0, 32 * q),
                    )
            # ACT cost scales with free size only -> exp the whole tile (every
            # row of col-tile q holds slot q's sum); the DMA, which does
            # support partition strides, picks rows {0,32,64,96}.
            affb = opool.tile([128, CHUNK], dt.float32, tag="affb")
            nc.scalar.activation(
                affb[:, :], pst[:, 0:CHUNK], ACTF.Exp, scale=-1.0 / 256.0
            )
            nc.sync.dma_start(
                outd[4 * t : 4 * t + 4, :], _pstride_view(affb[:], 4, CHUNK)
            )

    nc.compile()
    return nc


# ------------------------------ host wrapper ---------------------------------

_NC_CACHE = None
LAST_EXEC_NS = None
LAST_MEAN_EXEC_NS = None


def _get_nc():
    global _NC_CACHE
    if _NC_CACHE is None:
        _NC_CACHE = build_nc()
    return _NC_CACHE


def _prep_inputs(f2_in, f3_in, f4_in, w2, w3, w4, w9):
    """Shard/tile/cast on host. Returns per-core input maps."""

    def ctile(a, k):  # [C, S] -> [128, k*S] with c-tile t at cols [t*S, (t+1)*S)
        c, s = a.shape
        assert c == 128 * k
        return np.ascontiguousarray(
            a.reshape(k, 128, s).transpose(1, 0, 2).reshape(128, k * s)
        )

    w2t = ctile(np.asarray(w2, np.float32).T.astype(BF16), 4)      # [512,64]
    w3t = ctile(np.asarray(w3, np.float32).T.astype(BF16), 8)      # [1024,128]
    w4t = ctile(np.asarray(w4, np.float32).T.astype(BF16), 16)     # [2048,320]
    w9p = np.asarray(w9, np.float32)[:, XCAT_PERM].T               # [512 in, 512 out]
    w9p = w9p.copy()
    w9p[384:448, :] *= 1.0 / 64.0  # f2r rows: resize passes leave a 64x scale
    w9t = ctile(w9p.astype(BF16), 4)

    f2 = np.asarray(f2_in, np.float32).reshape(B, 512, 4096).astype(BF16)
    f3 = np.asarray(f3_in, np.float32).reshape(B, 1024, 1024).astype(BF16)
    f4 = np.asarray(f4_in, np.float32).reshape(B, 2048, 1024).astype(BF16)

    in_maps = []
    for b in range(B):
        in_maps.append(
            {
                "f2": ctile(f2[b], 4),
                "f3": ctile(f3[b], 8),
                "f4": ctile(f4[b], 16),
                "w2t": w2t,
                "w3t": w3t,
                "w4t": w4t,
                "w9t": w9t,
            }
        )
    return in_maps


def _install_trace_hooks():
    import types

    if "antenv.axon_hooks" not in sys.modules:
        mod = types.ModuleType("antenv.axon_hooks")
        _HOOK = [None]
        mod.set_axon_ntff_profile_hook = lambda h: _HOOK.__setitem__(0, h)
        mod.get_axon_ntff_profile_hook = lambda: _HOOK[0]
        sys.modules["antenv.axon_hooks"] = mod
        from trn_agent_boot.trn_boot import _ntff_profile_via_ctypes

        mod.set_axon_ntff_profile_hook(
            _ntff_profile_via_ctypes("/opt/axon/libaxon_pjrt.so")
        )
    import concourse.bass_utils as bass_utils

    bass_utils.upload_artifacts = lambda tmpdir: f"local:{tmpdir}"


def kernel(f2_in, f3_in, f4_in, w2, w3, w4, w9, _trace=False, _tmpdir=None):
    global LAST_EXEC_NS, LAST_MEAN_EXEC_NS
    from concourse.bass_utils import run_bass_kernel_spmd

    if _trace:
        _install_trace_hooks()

    nc = _get_nc()
    in_maps = _prep_inputs(f2_in, f3_in, f4_in, w2, w3, w4, w9)
    res = run_bass_kernel_spmd(
        nc, in_maps, list(range(N_CORES)), trace=_trace, tmpdir=_tmpdir
    )
    LAST_EXEC_NS = res.exec_time_ns
    LAST_MEAN_EXEC_NS = res.mean_exec_time_ns

    out = np.empty((B, 34, NPOS), np.float32)
    for b in range(B):
        flat = res.results[b]["aff"].reshape(-1)
        out[b] = flat[_FLAT_IDX]
    return out



# revision 20
# speedup vs baseline: 1.1435x; 1.1435x over previous
"""AffinityNet Trainium2 kernel: 8-core data-parallel (1 batch per core).

Per-core pipeline (batch b):
  f2 = elu(w2 @ f2_in[b])        [64, 64, 64]   (1x1 conv as matmul, bf16)
  f2r = bilinear_resize(f2)      [64, 32, 32]   (4-tap separable, DVE)
  f3 = elu(w3 @ f3_in[b])        [128, 32, 32]
  f4 = elu(w4 @ f4_in[b])        [320, 32, 32]
  x  = elu(w9p @ xcat)           [512, 32, 32]  (w9 host-permuted to the
                                                 on-chip channel layout)
  aff[p, i] = exp(-mean_c |x[c, to(p,i)] - x[c, from(i)]|)   [34, 672]

The neighbor gather is a pure 2D shift: done with overlapping strided AP
views on the vector engine (offsets grouped by (dy, dx-parity) into single
instructions); |.| on the scalar engine; channel sums via ones-matmul on PE
accumulating into per-slot PSUM rows at partitions {0,32,64,96}; the final
exp(-s/512) is fused into the PSUM->SBUF extraction on ACT. Host only
shards/reassembles and un-scrambles the slot layout.
"""

import sys

for _p in ("/opt/trn_rl_repo", "/root/.axon_site", "/root/.axon_site/_ro/pypackages"):
    if _p not in sys.path:
        sys.path.append(_p)

import numpy as np
import ml_dtypes

BF16 = ml_dtypes.bfloat16

# ---------------- problem constants (hardcoded from the spec) ----------------
B = 8
N_CORES = 8
CH = 28             # cropped height  (h range of "from" positions)
CW = 24             # cropped width   (w range of "from" positions)
NPOS = CH * CW      # 672

# offset list in the reference order (dy, dx)
OFFSETS = [(0, x) for x in range(1, 5)] + [
    (y, x) for y in range(1, 5) for x in range(-4, 5) if x * x + y * y < 25
]
assert len(OFFSETS) == 34

# groups: same dy, same-parity dx run (step 2) -> one DVE sub per (group, ctile)
GROUPS = [
    (0, [2, 4]),
    (0, [1, 3]),
    (1, [-4, -2, 0, 2, 4]),
    (1, [-3, -1, 1, 3]),
    (2, [-4, -2, 0, 2, 4]),
    (2, [-3, -1, 1, 3]),
    (3, [-2, 0, 2]),
    (3, [-3, -1, 1, 3]),
    (4, [-2, 0, 2]),
    (4, [-1, 1]),
]
assert sorted(set((dy, dx) for dy, dxs in GROUPS for dx in dxs)) == sorted(OFFSETS)

CHUNK = 336  # divides every group size evenly: 68 slots = 17 full banks

# slot list: (group, start col within group, size)
_SLOTS = []
for _g, (_dy, _dxs) in enumerate(GROUPS):
    _G = len(_dxs) * NPOS
    _c0 = 0
    while _c0 < _G:
        _cs = min(CHUNK, _G - _c0)
        _SLOTS.append((_g, _c0, _cs))
        _c0 += _cs
N_SLOTS = len(_SLOTS)                 # 68
N_BANKSEQ = N_SLOTS // 4              # 17
assert N_SLOTS % 4 == 0


def _build_unscramble():
    """aff[p, i] = out_flat[slot*CHUNK + col]; build (slot, col) index maps."""
    off_index = {od: i for i, od in enumerate(OFFSETS)}
    slot_of = np.zeros((34, NPOS), np.int64)
    col_of = np.zeros((34, NPOS), np.int64)
    slot_base = 0
    for g, (dy, dxs) in enumerate(GROUPS):
        G = len(dxs) * NPOS
        nchunks = (G + CHUNK - 1) // CHUNK
        for k, dx in enumerate(dxs):
            p = off_index[(dy, dx)]
            gcols = k * NPOS + np.arange(NPOS)
            slot_of[p, :] = slot_base + gcols // CHUNK
            col_of[p, :] = gcols % CHUNK
        slot_base += nchunks
    assert slot_base == N_SLOTS
    return slot_of, col_of


_SLOT_OF, _COL_OF = _build_unscramble()
_FLAT_IDX = _SLOT_OF * CHUNK + _COL_OF

# bilinear 64->32 resize taps (jax.image.resize, triangle kernel, antialias):
# interior: out[j] = .125 in[2j-1] + .375 in[2j] + .375 in[2j+1] + .125 in[2j+2]
# out[0] = (3 in[0] + 3 in[1] + in[2]) / 7 ; out[31] = (in[61] + 3 in[62] + 3 in[63]) / 7
W_IN = 0.375
W_OUT = 0.125
B3 = 3.0 / 7.0
B1 = 1.0 / 7.0

# xcat channel layout = [f3(128) | f4 m0(128) | f4 m1(128) | f2r(64)+f4 m2(64)];
# original concat order = [f2r 0:64 | f3 64:192 | f4 192:512]
XCAT_PERM = (
    list(range(64, 192))
    + list(range(192, 320))
    + list(range(320, 448))
    + list(range(0, 64))
    + list(range(448, 512))
)


# ------------------------------- bass kernel ---------------------------------

def _fview(t_ap, off, dims):
    """Strided free-dim view of a tile AP (partition dim preserved)."""
    import concourse.bass as bass

    pd = list(t_ap.ap)[0]
    return bass.AP(
        tensor=t_ap.tensor,
        offset=t_ap.offset + off,
        ap=[list(pd)] + [list(d) for d in dims],
    )


def _pstride_view(t_ap, nrows, ncols):
    """Partition-strided view: rows {0,32,64,...} of a [128, X] tile."""
    import concourse.bass as bass

    pd = list(t_ap.ap)[0]
    row_stride = pd[0]
    return bass.AP(
        tensor=t_ap.tensor,
        offset=t_ap.offset,
        ap=[[row_stride * 32, nrows], [1, ncols]],
    )


def build_nc():
    """Build + compile the per-core Bass program."""
    from contextlib import ExitStack

    import concourse.tile as tile
    from concourse import bacc, mybir

    dt = mybir.dt
    ALU = mybir.AluOpType
    ACTF = mybir.ActivationFunctionType

    nc = bacc.Bacc("TRN2", target_bir_lowering=False, debug=False)

    f2d = nc.dram_tensor("f2", [128, 4 * 4096], dt.bfloat16, kind="ExternalInput").ap()
    f3d = nc.dram_tensor("f3", [128, 8 * 1024], dt.bfloat16, kind="ExternalInput").ap()
    f4d = nc.dram_tensor("f4", [128, 16 * 1024], dt.bfloat16, kind="ExternalInput").ap()
    w2d = nc.dram_tensor("w2t", [128, 4 * 64], dt.bfloat16, kind="ExternalInput").ap()
    w3d = nc.dram_tensor("w3t", [128, 8 * 128], dt.bfloat16, kind="ExternalInput").ap()
    w4d = nc.dram_tensor("w4t", [128, 16 * 320], dt.bfloat16, kind="ExternalInput").ap()
    w9d = nc.dram_tensor("w9t", [128, 4 * 512], dt.bfloat16, kind="ExternalInput").ap()
    outd = nc.dram_tensor(
        "aff", [N_SLOTS, CHUNK], dt.float32, kind="ExternalOutput"
    ).ap()

    with tile.TileContext(nc) as tc, ExitStack() as ctx:
        wpool = ctx.enter_context(tc.tile_pool(name="w", bufs=1))
        fpool = ctx.enter_context(tc.tile_pool(name="fin", bufs=10))
        xpool = ctx.enter_context(tc.tile_pool(name="x", bufs=1))
        pspool = ctx.enter_context(tc.tile_pool(name="ps", bufs=8, space="PSUM"))
        tpool = ctx.enter_context(tc.tile_pool(name="tmp", bufs=4))
        dpool = ctx.enter_context(tc.tile_pool(name="diff", bufs=7))
        opool = ctx.enter_context(tc.tile_pool(name="out", bufs=4))

        # ---- constants; weights go on the scalar DMA queue, activations on
        # the sync queue. f3 first (smallest) so conv3 starts ASAP; f2 last
        # (conv2 runs after conv4 from resident SBUF).
        ones = wpool.tile([128, 32], dt.bfloat16, tag="ones")
        nc.vector.memset(ones[:], 1.0)
        scorr = wpool.tile([128, 1024], dt.bfloat16, tag="scorr")
        nc.gpsimd.memset(scorr[:], 0.0)

        w3sb = wpool.tile([128, 8 * 128], dt.bfloat16, tag="w3")
        nc.scalar.dma_start(w3sb[:], w3d[:])
        w4sb = wpool.tile([128, 16 * 320], dt.bfloat16, tag="w4")
        nc.scalar.dma_start(w4sb[:], w4d[:])
        w2sb = wpool.tile([128, 4 * 64], dt.bfloat16, tag="w2")
        nc.scalar.dma_start(w2sb[:], w2d[:])
        w9sb = wpool.tile([128, 4 * 512], dt.bfloat16, tag="w9")
        nc.scalar.dma_start(w9sb[:], w9d[:])

        f3c = [fpool.tile([128, 4096], dt.bfloat16, tag="fc", name=f"f3c{h}") for h in range(2)]
        for h in range(2):
            nc.sync.dma_start(f3c[h][:], f3d[:, h * 4096 : (h + 1) * 4096])
        f4c = [fpool.tile([128, 4096], dt.bfloat16, tag="fc", name=f"f4c{h}") for h in range(4)]
        for h in range(4):
            nc.sync.dma_start(f4c[h][:], f4d[:, h * 4096 : (h + 1) * 4096])
        f2c = [fpool.tile([128, 4096], dt.bfloat16, tag="fc", name=f"f2c{h}") for h in range(4)]
        for h in range(4):
            nc.sync.dma_start(f2c[h][:], f2d[:, h * 4096 : (h + 1) * 4096])

        xcat = xpool.tile([128, 4 * 1024], dt.bfloat16, tag="xcat")
        xbuf = xpool.tile([128, 4 * 1024], dt.bfloat16, tag="xbuf")
        xodd = xpool.tile([128, 4 * 1024], dt.bfloat16, tag="xodd")

        def elu(ps_t, pb, psz, dst_ap):
            """dst = elu(psum) = max(exp(min(x,0)) - 1, x): 2 ACT + 1 DVE."""
            ps_ap = ps_t[pb : pb + psz, :]
            u = tpool.tile([128, 512], dt.bfloat16, tag="eu", name="eu")
            e = tpool.tile([128, 512], dt.bfloat16, tag="ee", name="ee")
            nc.scalar.activation(u[pb : pb + psz, :], ps_ap, ACTF.Relu, scale=-1.0)
            nc.scalar.activation(
                e[pb : pb + psz, :], u[pb : pb + psz, :], ACTF.Exp, scale=-1.0
            )
            nc.vector.scalar_tensor_tensor(
                dst_ap, e[pb : pb + psz, :], -1.0, ps_ap, ALU.add, ALU.max
            )

        # ================= conv3 -> xcat[:, 0:1024] (ct0) =====================
        for n in range(2):
            ps3 = pspool.tile([128, 512], dt.float32, tag="ps", name=f"ps3_{n}")
            for k in range(8):
                nc.tensor.matmul(
                    ps3[:, :],
                    w3sb[:, k * 128 : (k + 1) * 128],
                    f3c[k // 4][:, (k % 4) * 1024 + n * 512 : (k % 4) * 1024 + (n + 1) * 512],
                    start=(k == 0),
                    stop=(k == 7),
                )
            elu(ps3, 0, 128, xcat[:, n * 512 : (n + 1) * 512])

        # ===== conv4 -> xcat ct1, ct2, ct3-rows-64:128 ========================
        # mi-outer: each m-chunk's elu overlaps the next chunk's matmuls.
        MCH4 = [(0, 128, 0), (128, 128, 0), (256, 64, 64)]  # (moff, msz, pbase)
        for mi, (moff, msz, pb) in enumerate(MCH4):
            ps4 = [
                pspool.tile([128, 512], dt.float32, tag="ps", name=f"ps4_{mi}_{n}")
                for n in range(2)
            ]
            for k in range(16):
                for n in range(2):
                    nc.tensor.matmul(
                        ps4[n][pb : pb + msz, :],
                        w4sb[:, k * 320 + moff : k * 320 + moff + msz],
                        f4c[k // 4][:, (k % 4) * 1024 + n * 512 : (k % 4) * 1024 + (n + 1) * 512],
                        start=(k == 0),
                        stop=(k == 15),
                        tile_position=(0, pb),
                    )
            for n in range(2):
                dst = xcat[pb : pb + msz, (1 + mi) * 1024 + n * 512 : (1 + mi) * 1024 + (n + 1) * 512]
                elu(ps4[n], pb, msz, dst)

        # ========== conv2 (2-way col-tiled, M=64): x2 = elu(w2 @ f2) ==========
        # x2 split layout [128, 2048]: partitions 64*w_+c hold image half
        # h in [32*w_, 32*w_+32), flat col = 64*(h%32) + w.
        x2 = xpool.tile([128, 2048], dt.bfloat16, tag="x2")
        for j in range(4):
            ps2 = pspool.tile([128, 512], dt.float32, tag="ps", name=f"ps2_{j}")
            for k in range(4):
                for w_ in range(2):
                    n = w_ * 4 + j
                    nc.tensor.matmul(
                        ps2[64 * w_ : 64 * w_ + 64, :],
                        w2sb[:, k * 64 : (k + 1) * 64],
                        f2c[k][:, n * 512 : (n + 1) * 512],
                        start=(k == 0),
                        stop=(k == 3),
                        tile_position=(0, 64 * w_),
                    )
            for w_ in range(2):
                elu(ps2, 64 * w_, 64, x2[64 * w_ : 64 * w_ + 64, 512 * j : 512 * (j + 1)])

        # ================= resize: x2 -> xcat[0:64, 3072:4096] ================
        # Pairwise-sum form, unnormalized by 8x per pass (64x total, divided
        # out of w9's f2r columns on the host):
        #   W': A[j]=in[2j]+in[2j+1], B[j]=in[2j-1]+in[2j+2], rw2=3A+B
        #   H': C[i]=r[2i]+r[2i+1],  D[i]=r[2i-1]+r[2i+2],  out=3C+D
        # Boundary taps (j,i in {0,31}): out=(24*pair + 8*third)/7.
        # DVE ops need src/dst at the same base partition, so the W' result
        # stays in the split layout rw [128, 32hh x 32j]; a DMA folds the
        # upper image half down to rwup [64, 1024] for the H pass.
        F2R0 = 3072
        xc64 = xcat[0:64, :]
        x2a = x2[:]
        rA = xpool.tile([128, 1024], dt.bfloat16, tag="rA")
        rB = xpool.tile([128, 1024], dt.bfloat16, tag="rB")
        rw = xpool.tile([128, 1024], dt.bfloat16, tag="rw")
        rwup = xpool.tile([64, 1024], dt.bfloat16, tag="rwup")
        nc.vector.tensor_tensor(
            _fview(rA[:], 0, [(32, 32), (1, 32)]),
            _fview(x2a, 0, [(64, 32), (2, 32)]),
            _fview(x2a, 1, [(64, 32), (2, 32)]),
            ALU.add,
        )
        nc.vector.tensor_tensor(
            _fview(rB[:], 1, [(32, 32), (1, 30)]),
            _fview(x2a, 1, [(64, 32), (2, 30)]),
            _fview(x2a, 4, [(64, 32), (2, 30)]),
            ALU.add,
        )
        rwa = rw[:]
        nc.vector.scalar_tensor_tensor(
            _fview(rwa, 1, [(32, 32), (1, 30)]),
            _fview(rA[:], 1, [(32, 32), (1, 30)]), 3.0,
            _fview(rB[:], 1, [(32, 32), (1, 30)]), ALU.mult, ALU.add,
        )
        rj0 = _fview(rwa, 0, [(32, 32), (1, 1)])
        nc.vector.tensor_scalar_mul(rj0, _fview(rA[:], 0, [(32, 32), (1, 1)]), 24.0 / 7.0)
        nc.vector.scalar_tensor_tensor(
            rj0, _fview(x2a, 2, [(64, 32), (1, 1)]), 8.0 / 7.0, rj0, ALU.mult, ALU.add
        )
        rj31 = _fview(rwa, 31, [(32, 32), (1, 1)])
        nc.vector.tensor_scalar_mul(rj31, _fview(rA[:], 31, [(32, 32), (1, 1)]), 24.0 / 7.0)
        nc.vector.scalar_tensor_tensor(
            rj31, _fview(x2a, 61, [(64, 32), (1, 1)]), 8.0 / 7.0, rj31, ALU.mult, ALU.add
        )
        nc.sync.dma_start(rwup[:], rw[64:128, :])
        rlo = rw[0:64, :]
        rup = rwup[:]
        rC = xpool.tile([64, 1024], dt.bfloat16, tag="rC")
        rD = xpool.tile([64, 1024], dt.bfloat16, tag="rD")
        # C[i] = r[2i] + r[2i+1]: i 0..15 from rlo, 16..31 from rup
        nc.vector.tensor_tensor(
            _fview(rC[:], 0, [(32, 16), (1, 32)]),
            _fview(rlo, 0, [(64, 16), (1, 32)]),
            _fview(rlo, 32, [(64, 16), (1, 32)]), ALU.add,
        )
        nc.vector.tensor_tensor(
            _fview(rC[:], 512, [(32, 16), (1, 32)]),
            _fview(rup, 0, [(64, 16), (1, 32)]),
            _fview(rup, 32, [(64, 16), (1, 32)]), ALU.add,
        )
        # D[i] = r[2i-1] + r[2i+2]: i 1..14 lo, 15/16 cross, 17..30 up
        nc.vector.tensor_tensor(
            _fview(rD[:], 32, [(32, 14), (1, 32)]),
            _fview(rlo, 32, [(64, 14), (1, 32)]),
            _fview(rlo, 128, [(64, 14), (1, 32)]), ALU.add,
        )
        nc.vector.tensor_tensor(
            _fview(rD[:], 15 * 32, [(32, 1), (1, 32)]),
            _fview(rlo, 29 * 32, [(32, 1), (1, 32)]),
            _fview(rup, 0, [(32, 1), (1, 32)]), ALU.add,
        )
        nc.vector.tensor_tensor(
            _fview(rD[:], 16 * 32, [(32, 1), (1, 32)]),
            _fview(rlo, 31 * 32, [(32, 1), (1, 32)]),
            _fview(rup, 2 * 32, [(32, 1), (1, 32)]), ALU.add,
        )
        nc.vector.tensor_tensor(
            _fview(rD[:], 17 * 32, [(32, 14), (1, 32)]),
            _fview(rup, 32, [(64, 14), (1, 32)]),
            _fview(rup, 128, [(64, 14), (1, 32)]), ALU.add,
        )
        nc.vector.scalar_tensor_tensor(
            _fview(xc64, F2R0 + 32, [(32, 30), (1, 32)]),
            _fview(rC[:], 32, [(32, 30), (1, 32)]), 3.0,
            _fview(rD[:], 32, [(32, 30), (1, 32)]), ALU.mult, ALU.add,
        )
        oi0 = _fview(xc64, F2R0, [(32, 1), (1, 32)])
        nc.vector.tensor_scalar_mul(oi0, _fview(rC[:], 0, [(32, 1), (1, 32)]), 24.0 / 7.0)
        nc.vector.scalar_tensor_tensor(
            oi0, _fview(rlo, 2 * 32, [(32, 1), (1, 32)]), 8.0 / 7.0, oi0, ALU.mult, ALU.add
        )
        oi31 = _fview(xc64, F2R0 + 31 * 32, [(32, 1), (1, 32)])
        nc.vector.tensor_scalar_mul(oi31, _fview(rC[:], 31 * 32, [(32, 1), (1, 32)]), 24.0 / 7.0)
        nc.vector.scalar_tensor_tensor(
            oi31, _fview(rup, 29 * 32, [(32, 1), (1, 32)]), 8.0 / 7.0, oi31, ALU.mult, ALU.add
        )

        # ================= conv9 -> xbuf ======================================
        # k-outer: xcat ct0..2 are ready before ct3 (resize tail), so 24 of
        # the 32 matmuls can stream while the resize finishes.
        ps9 = [
            pspool.tile([128, 512], dt.float32, tag="ps", name=f"ps9_{m}_{n}")
            for m in range(4) for n in range(2)
        ]
        for k in range(4):
            for m in range(4):
                for n in range(2):
                    nc.tensor.matmul(
                        ps9[m * 2 + n][:, :],
                        w9sb[:, k * 512 + m * 128 : k * 512 + (m + 1) * 128],
                        xcat[:, k * 1024 + n * 512 : k * 1024 + (n + 1) * 512],
                        start=(k == 0),
                        stop=(k == 3),
                    )
        for m in range(4):
            for n in range(2):
                elu(ps9[m * 2 + n], 0, 128, xbuf[:, m * 1024 + n * 512 : m * 1024 + (n + 1) * 512])

        # xodd[p, j] = xbuf[p, j+1]  (for 4B-aligned odd-dx views); per-ctile
        # DMA shift-copies keep it off the DVE and start as soon as each
        # conv9 m-tile lands.
        for ct in range(4):
            nc.sync.dma_start(
                xodd[:, ct * 1024 : ct * 1024 + 1023],
                xbuf[:, ct * 1024 + 1 : ct * 1024 + 1024],
            )

        # S[pos] = sum_c x[c, pos] (rows 0-31 identical); s_corr = -S/2 in bf16
        ps_s = [
            pspool.tile([128, 512], dt.float32, tag="ps", name=f"ps_s{n}")
            for n in range(2)
        ]
        for n in range(2):
            for ct in range(4):
                nc.tensor.matmul(
                    ps_s[n][0:32, :],
                    ones[:, 0:32],
                    xbuf[:, ct * 1024 + n * 512 : ct * 1024 + (n + 1) * 512],
                    start=(ct == 0),
                    stop=(ct == 3),
                )
        # s_corr rows: hi = bf16(-S/2), lo = residual (-S/2 - hi); k=2 matmul
        # sums both, recovering ~fp32 accuracy from bf16 operands.
        s_tmp = xpool.tile([1, 2048], dt.bfloat16, tag="s_tmp")
        for n in range(2):
            nc.scalar.activation(
                s_tmp[0:1, n * 512 : (n + 1) * 512], ps_s[n][0:1, :], ACTF.Copy, scale=-0.5
            )
            nc.vector.scalar_tensor_tensor(
                s_tmp[0:1, 1024 + n * 512 : 1024 + (n + 1) * 512],
                ps_s[n][0:1, :], -0.5,
                s_tmp[0:1, n * 512 : (n + 1) * 512],
                ALU.mult, ALU.subtract,
            )
        # scorr rows 0/1: hi = bf16(-S/2), lo = residual; rows 2..127 zero so
        # the correction matmuls can be K=128 (same (128,32) PE tile mode and
        # same all-ones lhsT as the channel-sum matmuls -> no mode switches).
        nc.sync.dma_start(scorr[0:1, :], s_tmp[0:1, 0:1024])
        nc.sync.dma_start(scorr[1:2, :], s_tmp[0:1, 1024:2048])

        # ================= affinity ==========================================
        # Per bank-quad: 4 slots run on the 4 (128,32) PE column tiles
        # concurrently (interleaved issue), 6 chained K=128 matmuls each
        # (4 channel-sum + 2 corrections vs zero-padded scorr). One ACT exp
        # over the partition-strided rows {0,32,64,96} extracts the quad.
        xba = xbuf[:]
        atiles_by_group = {}

        def ensure_group(g):
            if g in atiles_by_group:
                return
            dy, dxs = GROUPS[g]
            ndx = len(dxs)
            G = ndx * NPOS
            odd = dxs[0] % 2 != 0
            tiles = []
            for ct in range(4):
                ctb = ct * 1024
                ff = _fview(xba, ctb + 4, [(0, ndx), (32, CH), (1, CW)])
                if odd:
                    ft = _fview(
                        xodd[:], ctb + 32 * dy + 4 + dxs[0] - 1,
                        [(2, ndx), (32, CH), (1, CW)],
                    )
                else:
                    ft = _fview(
                        xba, ctb + 32 * dy + 4 + dxs[0],
                        [(2, ndx), (32, CH), (1, CW)],
                    )
                mtile = dpool.tile([128, G], dt.bfloat16, tag="d", name=f"m_{g}_{ct}")
                mv = _fview(mtile[:], 0, [(NPOS, ndx), (CW, CH), (1, CW)])
                nc.vector.tensor_tensor(mv, ft, ff, ALU.max)
                tiles.append(mtile)
            atiles_by_group[g] = tiles

        for t in range(N_BANKSEQ):
            slots = list(range(4 * t, 4 * t + 4))
            for s in slots:
                ensure_group(_SLOTS[s][0])
            pst = pspool.tile([128, 512], dt.float32, tag="ps", name=f"pq_{t}")
            for k in range(6):
                for q, s in enumerate(slots):
                    g, c0, cs = _SLOTS[s]
                    dy, dxs = GROUPS[g]
                    if k < 4:
                        rhs = atiles_by_group[g][k][:, c0 : c0 + cs]
                    else:
                        dxi, h2 = (c0 // NPOS), (c0 % NPOS) // 336
                        off = 4 + 32 * 14 * h2
                        if k == 5:
                            off += 32 * dy + dxs[dxi]
                        rhs = _fview(scorr[:], off, [(32, 14), (1, CW)])
                    nc.tensor.matmul(
                        pst[32 * q : 32 * q + 32, 0:cs],
                        ones[:, 0:32],
                        rhs,
                        start=(k == 0),
                        stop=(k == 5),
                        tile_position=(0, 32 * q),
                    )
            # ACT cost scales with free size only -> exp the whole tile (every
            # row of col-tile q holds slot q's sum); the DMA, which does
            # support partition strides, picks rows {0,32,64,96}.
            affb = opool.tile([128, CHUNK], dt.float32, tag="affb")
            nc.scalar.activation(
                affb[:, :], pst[:, 0:CHUNK], ACTF.Exp, scale=-1.0 / 256.0
            )
            nc.sync.dma_start(
                outd[4 * t : 4 * t + 4, :], _pstride_view(affb[:], 4, CHUNK)
            )

    nc.compile()
    return nc


# ------------------------------ host wrapper ---------------------------------

_NC_CACHE = None
LAST_EXEC_NS = None
LAST_MEAN_EXEC_NS = None


def _get_nc():
    global _NC_CACHE
    if _NC_CACHE is None:
        _NC_CACHE = build_nc()
    return _NC_CACHE


def _prep_inputs(f2_in, f3_in, f4_in, w2, w3, w4, w9):
    """Shard/tile/cast on host. Returns per-core input maps."""

    def ctile(a, k):  # [C, S] -> [128, k*S] with c-tile t at cols [t*S, (t+1)*S)
        c, s = a.shape
        assert c == 128 * k
        return np.ascontiguousarray(
            a.reshape(k, 128, s).transpose(1, 0, 2).reshape(128, k * s)
        )

    w2t = ctile(np.asarray(w2, np.float32).T.astype(BF16), 4)      # [512,64]
    w3t = ctile(np.asarray(w3, np.float32).T.astype(BF16), 8)      # [1024,128]
    w4t = ctile(np.asarray(w4, np.float32).T.astype(BF16), 16)     # [2048,320]
    w9p = np.asarray(w9, np.float32)[:, XCAT_PERM].T               # [512 in, 512 out]
    w9p = w9p.copy()
    w9p[384:448, :] *= 1.0 / 64.0  # f2r rows: resize passes leave a 64x scale
    w9t = ctile(w9p.astype(BF16), 4)

    f2 = np.asarray(f2_in, np.float32).reshape(B, 512, 4096).astype(BF16)
    f3 = np.asarray(f3_in, np.float32).reshape(B, 1024, 1024).astype(BF16)
    f4 = np.asarray(f4_in, np.float32).reshape(B, 2048, 1024).astype(BF16)

    in_maps = []
    for b in range(B):
        in_maps.append(
            {
                "f2": ctile(f2[b], 4),
                "f3": ctile(f3[b], 8),
                "f4": ctile(f4[b], 16),
                "w2t": w2t,
                "w3t": w3t,
                "w4t": w4t,
                "w9t": w9t,
            }
        )
    return in_maps


def _install_trace_hooks():
    import types

    if "antenv.axon_hooks" not in sys.modules:
        mod = types.ModuleType("antenv.axon_hooks")
        _HOOK = [None]
        mod.set_axon_ntff_profile_hook = lambda h: _HOOK.__setitem__(0, h)
        mod.get_axon_ntff_profile_hook = lambda: _HOOK[0]
        sys.modules["antenv.axon_hooks"] = mod
        from trn_agent_boot.trn_boot import _ntff_profile_via_ctypes

        mod.set_axon_ntff_profile_hook(
            _ntff_profile_via_ctypes("/opt/axon/libaxon_pjrt.so")
        )
    import concourse.bass_utils as bass_utils

    bass_utils.upload_artifacts = lambda tmpdir: f"local:{tmpdir}"


def kernel(f2_in, f3_in, f4_in, w2, w3, w4, w9, _trace=False, _tmpdir=None):
    global LAST_EXEC_NS, LAST_MEAN_EXEC_NS
    from concourse.bass_utils import run_bass_kernel_spmd

    if _trace:
        _install_trace_hooks()

    nc = _get_nc()
    in_maps = _prep_inputs(f2_in, f3_in, f4_in, w2, w3, w4, w9)
    res = run_bass_kernel_spmd(
        nc, in_maps, list(range(N_CORES)), trace=_trace, tmpdir=_tmpdir
    )
    LAST_EXEC_NS = res.exec_time_ns
    LAST_MEAN_EXEC_NS = res.mean_exec_time_ns

    out = np.empty((B, 34, NPOS), np.float32)
    for b in range(B):
        flat = res.results[b]["aff"].reshape(-1)
        out[b] = flat[_FLAT_IDX]
    return out



# revision 27
# speedup vs baseline: 1.1719x; 1.0248x over previous
"""AffinityNet Trainium2 kernel: 8-core data-parallel (1 batch per core).

Per-core pipeline (batch b):
  f2 = elu(w2 @ f2_in[b])        [64, 64, 64]   (1x1 conv as matmul, bf16)
  f2r = bilinear_resize(f2)      [64, 32, 32]   (4-tap separable, DVE)
  f3 = elu(w3 @ f3_in[b])        [128, 32, 32]
  f4 = elu(w4 @ f4_in[b])        [320, 32, 32]
  x  = elu(w9p @ xcat)           [512, 32, 32]  (w9 host-permuted to the
                                                 on-chip channel layout)
  aff[p, i] = exp(-mean_c |x[c, to(p,i)] - x[c, from(i)]|)   [34, 672]

The neighbor gather is a pure 2D shift: done with overlapping strided AP
views on the vector engine (offsets grouped by (dy, dx-parity) into single
instructions); |.| on the scalar engine; channel sums via ones-matmul on PE
accumulating into per-slot PSUM rows at partitions {0,32,64,96}; the final
exp(-s/512) is fused into the PSUM->SBUF extraction on ACT. Host only
shards/reassembles and un-scrambles the slot layout.
"""

import sys

for _p in ("/opt/trn_rl_repo", "/root/.axon_site", "/root/.axon_site/_ro/pypackages"):
    if _p not in sys.path:
        sys.path.append(_p)

import numpy as np
import ml_dtypes

BF16 = ml_dtypes.bfloat16

# ---------------- problem constants (hardcoded from the spec) ----------------
B = 8
N_CORES = 8
CH = 28             # cropped height  (h range of "from" positions)
CW = 24             # cropped width   (w range of "from" positions)
NPOS = CH * CW      # 672

# offset list in the reference order (dy, dx)
OFFSETS = [(0, x) for x in range(1, 5)] + [
    (y, x) for y in range(1, 5) for x in range(-4, 5) if x * x + y * y < 25
]
assert len(OFFSETS) == 34

# groups: same dy, same-parity dx run (step 2) -> one DVE sub per (group, ctile)
GROUPS = [
    (0, [2, 4]),
    (0, [1, 3]),
    (1, [-4, -2, 0, 2, 4]),
    (1, [-3, -1, 1, 3]),
    (2, [-4, -2, 0, 2, 4]),
    (2, [-3, -1, 1, 3]),
    (3, [-2, 0, 2]),
    (3, [-3, -1, 1, 3]),
    (4, [-2, 0, 2]),
    (4, [-1, 1]),
]
assert sorted(set((dy, dx) for dy, dxs in GROUPS for dx in dxs)) == sorted(OFFSETS)

CHUNK = 336  # divides every group size evenly: 68 slots = 17 full banks

# slot list: (group, start col within group, size)
_SLOTS = []
for _g, (_dy, _dxs) in enumerate(GROUPS):
    _G = len(_dxs) * NPOS
    _c0 = 0
    while _c0 < _G:
        _cs = min(CHUNK, _G - _c0)
        _SLOTS.append((_g, _c0, _cs))
        _c0 += _cs
N_SLOTS = len(_SLOTS)                 # 68
N_BANKSEQ = N_SLOTS // 4              # 17
assert N_SLOTS % 4 == 0


def _build_unscramble():
    """aff[p, i] = out_flat[slot*CHUNK + col]; build (slot, col) index maps."""
    off_index = {od: i for i, od in enumerate(OFFSETS)}
    slot_of = np.zeros((34, NPOS), np.int64)
    col_of = np.zeros((34, NPOS), np.int64)
    slot_base = 0
    for g, (dy, dxs) in enumerate(GROUPS):
        G = len(dxs) * NPOS
        nchunks = (G + CHUNK - 1) // CHUNK
        for k, dx in enumerate(dxs):
            p = off_index[(dy, dx)]
            gcols = k * NPOS + np.arange(NPOS)
            slot_of[p, :] = slot_base + gcols // CHUNK
            col_of[p, :] = gcols % CHUNK
        slot_base += nchunks
    assert slot_base == N_SLOTS
    return slot_of, col_of


_SLOT_OF, _COL_OF = _build_unscramble()
_FLAT_IDX = _SLOT_OF * CHUNK + _COL_OF

# bilinear 64->32 resize taps (jax.image.resize, triangle kernel, antialias):
# interior: out[j] = .125 in[2j-1] + .375 in[2j] + .375 in[2j+1] + .125 in[2j+2]
# out[0] = (3 in[0] + 3 in[1] + in[2]) / 7 ; out[31] = (in[61] + 3 in[62] + 3 in[63]) / 7
W_IN = 0.375
W_OUT = 0.125
B3 = 3.0 / 7.0
B1 = 1.0 / 7.0

# xcat channel layout = [f3(128) | f4 m0(128) | f4 m1(128) | f2r(64)+f4 m2(64)];
# original concat order = [f2r 0:64 | f3 64:192 | f4 192:512]
XCAT_PERM = (
    list(range(64, 192))
    + list(range(192, 320))
    + list(range(320, 448))
    + list(range(0, 64))
    + list(range(448, 512))
)


# ------------------------------- bass kernel ---------------------------------

def _fview(t_ap, off, dims):
    """Strided free-dim view of a tile AP (partition dim preserved)."""
    import concourse.bass as bass

    pd = list(t_ap.ap)[0]
    return bass.AP(
        tensor=t_ap.tensor,
        offset=t_ap.offset + off,
        ap=[list(pd)] + [list(d) for d in dims],
    )


def _pstride_view(t_ap, nrows, ncols):
    """Partition-strided view: rows {0,32,64,...} of a [128, X] tile."""
    import concourse.bass as bass

    pd = list(t_ap.ap)[0]
    row_stride = pd[0]
    return bass.AP(
        tensor=t_ap.tensor,
        offset=t_ap.offset,
        ap=[[row_stride * 32, nrows], [1, ncols]],
    )


def build_nc():
    """Build + compile the per-core Bass program."""
    from contextlib import ExitStack

    import concourse.tile as tile
    from concourse import bacc, mybir

    dt = mybir.dt
    ALU = mybir.AluOpType
    ACTF = mybir.ActivationFunctionType

    nc = bacc.Bacc("TRN2", target_bir_lowering=False, debug=False)

    f2d = nc.dram_tensor("f2", [128, 4 * 4096], dt.bfloat16, kind="ExternalInput").ap()
    f3d = nc.dram_tensor("f3", [128, 8 * 1024], dt.bfloat16, kind="ExternalInput").ap()
    f4d = nc.dram_tensor("f4", [128, 16 * 1024], dt.bfloat16, kind="ExternalInput").ap()
    w2d = nc.dram_tensor("w2t", [128, 4 * 64], dt.bfloat16, kind="ExternalInput").ap()
    w3d = nc.dram_tensor("w3t", [128, 8 * 128], dt.bfloat16, kind="ExternalInput").ap()
    w4d = nc.dram_tensor("w4t", [128, 16 * 320], dt.bfloat16, kind="ExternalInput").ap()
    w9d = nc.dram_tensor("w9t", [128, 4 * 512], dt.bfloat16, kind="ExternalInput").ap()
    outd = nc.dram_tensor(
        "aff", [N_SLOTS, CHUNK], dt.float32, kind="ExternalOutput"
    ).ap()

    with tile.TileContext(nc) as tc, ExitStack() as ctx:
        wpool = ctx.enter_context(tc.tile_pool(name="w", bufs=1))
        fpool = ctx.enter_context(tc.tile_pool(name="fin", bufs=10))
        xpool = ctx.enter_context(tc.tile_pool(name="x", bufs=1))
        pspool = ctx.enter_context(tc.tile_pool(name="ps", bufs=8, space="PSUM"))
        tpool = ctx.enter_context(tc.tile_pool(name="tmp", bufs=4))
        dpool = ctx.enter_context(tc.tile_pool(name="diff", bufs=7))
        opool = ctx.enter_context(tc.tile_pool(name="out", bufs=2))

        # ---- constants; weights go on the scalar DMA queue, activations on
        # the sync queue. f3 first (smallest) so conv3 starts ASAP; f2 last
        # (conv2 runs after conv4 from resident SBUF).
        ones = wpool.tile([128, 32], dt.bfloat16, tag="ones")
        nc.vector.memset(ones[:], 1.0)
        scorr = wpool.tile([128, 1024], dt.bfloat16, tag="scorr")
        nc.gpsimd.memset(scorr[:], 0.0)

        w3sb = wpool.tile([128, 8 * 128], dt.bfloat16, tag="w3")
        nc.scalar.dma_start(w3sb[:], w3d[:])
        w4sb = wpool.tile([128, 16 * 320], dt.bfloat16, tag="w4")
        nc.scalar.dma_start(w4sb[:], w4d[:])
        w2sb = wpool.tile([128, 4 * 64], dt.bfloat16, tag="w2")
        nc.scalar.dma_start(w2sb[:], w2d[:])
        w9sb = wpool.tile([128, 4 * 512], dt.bfloat16, tag="w9")
        nc.scalar.dma_start(w9sb[:], w9d[:])

        f3c = [fpool.tile([128, 4096], dt.bfloat16, tag="fc", name=f"f3c{h}") for h in range(2)]
        for h in range(2):
            nc.sync.dma_start(f3c[h][:], f3d[:, h * 4096 : (h + 1) * 4096])
        f4c = [fpool.tile([128, 4096], dt.bfloat16, tag="fc", name=f"f4c{h}") for h in range(4)]
        for h in range(4):
            nc.sync.dma_start(f4c[h][:], f4d[:, h * 4096 : (h + 1) * 4096])
        f2c = [fpool.tile([128, 4096], dt.bfloat16, tag="fc", name=f"f2c{h}") for h in range(4)]
        for h in range(4):
            nc.sync.dma_start(f2c[h][:], f2d[:, h * 4096 : (h + 1) * 4096])

        xcat = xpool.tile([128, 4 * 1024], dt.bfloat16, tag="xcat")
        xbuf = xpool.tile([128, 4 * 1024], dt.bfloat16, tag="xbuf")
        xodd = xpool.tile([128, 4 * 1024], dt.bfloat16, tag="xodd")

        def elu(ps_t, pb, psz, dst_ap, pool_stt=False):
            """dst = elu(psum). Default: 2 ACT + DVE max(exp(min(x,0))-1, x).
            pool_stt: 3 ACT + the combine on the (otherwise idle) Pool engine,
            which cannot read PSUM -> needs the extra Relu extraction."""
            ps_ap = ps_t[pb : pb + psz, :]
            u = tpool.tile([128, 512], dt.bfloat16, tag="eu", name="eu")
            e = tpool.tile([128, 512], dt.bfloat16, tag="ee", name="ee")
            nc.scalar.activation(u[pb : pb + psz, :], ps_ap, ACTF.Relu, scale=-1.0)
            nc.scalar.activation(
                e[pb : pb + psz, :], u[pb : pb + psz, :], ACTF.Exp, scale=-1.0
            )
            if pool_stt:
                # Pool has no scalar_tensor_tensor; em1 = e-1 on ACT (exp's
                # affine is pre-lookup, so -1 needs its own op), then a plain
                # Pool tensor_tensor max against the Relu extraction.
                r = tpool.tile([128, 512], dt.bfloat16, tag="er", name="er")
                nc.scalar.activation(r[pb : pb + psz, :], ps_ap, ACTF.Relu)
                em1 = tpool.tile([128, 512], dt.bfloat16, tag="em", name="em")
                nc.scalar.activation(
                    em1[pb : pb + psz, :], e[pb : pb + psz, :], ACTF.Copy, bias=-1.0
                )
                nc.gpsimd.tensor_tensor(
                    dst_ap, em1[pb : pb + psz, :], r[pb : pb + psz, :], ALU.add
                )
            else:
                nc.vector.scalar_tensor_tensor(
                    dst_ap, e[pb : pb + psz, :], -1.0, ps_ap, ALU.add, ALU.max
                )

        # ================= conv3 -> xcat[:, 0:1024] (ct0) =====================
        for n in range(2):
            ps3 = pspool.tile([128, 512], dt.float32, tag="ps", name=f"ps3_{n}")
            for k in range(8):
                nc.tensor.matmul(
                    ps3[:, :],
                    w3sb[:, k * 128 : (k + 1) * 128],
                    f3c[k // 4][:, (k % 4) * 1024 + n * 512 : (k % 4) * 1024 + (n + 1) * 512],
                    start=(k == 0),
                    stop=(k == 7),
                )
            elu(ps3, 0, 128, xcat[:, n * 512 : (n + 1) * 512])

        # ===== conv4 -> xcat ct1, ct2, ct3-rows-64:128 ========================
        # mi-outer: each m-chunk's elu overlaps the next chunk's matmuls.
        MCH4 = [(0, 128, 0), (128, 128, 0), (256, 64, 64)]  # (moff, msz, pbase)
        for mi, (moff, msz, pb) in enumerate(MCH4):
            ps4 = [
                pspool.tile([128, 512], dt.float32, tag="ps", name=f"ps4_{mi}_{n}")
                for n in range(2)
            ]
            for k in range(16):
                for n in range(2):
                    nc.tensor.matmul(
                        ps4[n][pb : pb + msz, :],
                        w4sb[:, k * 320 + moff : k * 320 + moff + msz],
                        f4c[k // 4][:, (k % 4) * 1024 + n * 512 : (k % 4) * 1024 + (n + 1) * 512],
                        start=(k == 0),
                        stop=(k == 15),
                        tile_position=(0, pb),
                    )
            for n in range(2):
                dst = xcat[pb : pb + msz, (1 + mi) * 1024 + n * 512 : (1 + mi) * 1024 + (n + 1) * 512]
                elu(ps4[n], pb, msz, dst)

        # ========== conv2 (2-way col-tiled, M=64): x2 = elu(w2 @ f2) ==========
        # x2 split layout [128, 2048]: partitions 64*w_+c hold image half
        # h in [32*w_, 32*w_+32), flat col = 64*(h%32) + w.
        x2 = xpool.tile([128, 2048], dt.bfloat16, tag="x2")
        for j in range(4):
            ps2 = pspool.tile([128, 512], dt.float32, tag="ps", name=f"ps2_{j}")
            for k in range(4):
                for w_ in range(2):
                    n = w_ * 4 + j
                    nc.tensor.matmul(
                        ps2[64 * w_ : 64 * w_ + 64, :],
                        w2sb[:, k * 64 : (k + 1) * 64],
                        f2c[k][:, n * 512 : (n + 1) * 512],
                        start=(k == 0),
                        stop=(k == 3),
                        tile_position=(0, 64 * w_),
                    )
            elu(ps2, 0, 128, x2[:, 512 * j : 512 * (j + 1)])

        # ================= resize: x2 -> xcat[0:64, 3072:4096] ================
        # Pairwise-sum form, unnormalized by 8x per pass (64x total, divided
        # out of w9's f2r columns on the host):
        #   W': A[j]=in[2j]+in[2j+1], B[j]=in[2j-1]+in[2j+2], rw2=3A+B
        #   H': C[i]=r[2i]+r[2i+1],  D[i]=r[2i-1]+r[2i+2],  out=3C+D
        # Boundary taps (j,i in {0,31}): out=(24*pair + 8*third)/7.
        # DVE ops need src/dst at the same base partition, so the W' result
        # stays in the split layout rw [128, 32hh x 32j]; a DMA folds the
        # upper image half down to rwup [64, 1024] for the H pass.
        F2R0 = 3072
        xc64 = xcat[0:64, :]
        x2a = x2[:]
        rA = xpool.tile([128, 1024], dt.bfloat16, tag="rA")
        rB = xpool.tile([128, 1024], dt.bfloat16, tag="rB")
        rw = xpool.tile([128, 1024], dt.bfloat16, tag="rw")
        rwup = xpool.tile([64, 1024], dt.bfloat16, tag="rwup")
        nc.vector.tensor_tensor(
            _fview(rA[:], 0, [(32, 32), (1, 32)]),
            _fview(x2a, 0, [(64, 32), (2, 32)]),
            _fview(x2a, 1, [(64, 32), (2, 32)]),
            ALU.add,
        )
        nc.vector.tensor_tensor(
            _fview(rB[:], 1, [(32, 32), (1, 30)]),
            _fview(x2a, 1, [(64, 32), (2, 30)]),
            _fview(x2a, 4, [(64, 32), (2, 30)]),
            ALU.add,
        )
        rwa = rw[:]
        nc.vector.scalar_tensor_tensor(
            _fview(rwa, 1, [(32, 32), (1, 30)]),
            _fview(rA[:], 1, [(32, 32), (1, 30)]), 3.0,
            _fview(rB[:], 1, [(32, 32), (1, 30)]), ALU.mult, ALU.add,
        )
        rj0 = _fview(rwa, 0, [(32, 32), (1, 1)])
        nc.vector.tensor_scalar_mul(rj0, _fview(rA[:], 0, [(32, 32), (1, 1)]), 24.0 / 7.0)
        nc.vector.scalar_tensor_tensor(
            rj0, _fview(x2a, 2, [(64, 32), (1, 1)]), 8.0 / 7.0, rj0, ALU.mult, ALU.add
        )
        rj31 = _fview(rwa, 31, [(32, 32), (1, 1)])
        nc.vector.tensor_scalar_mul(rj31, _fview(rA[:], 31, [(32, 32), (1, 1)]), 24.0 / 7.0)
        nc.vector.scalar_tensor_tensor(
            rj31, _fview(x2a, 61, [(64, 32), (1, 1)]), 8.0 / 7.0, rj31, ALU.mult, ALU.add
        )
        nc.sync.dma_start(rwup[:], rw[64:128, :])
        rlo = rw[0:64, :]
        rup = rwup[:]
        rC = xpool.tile([64, 1024], dt.bfloat16, tag="rC")
        rD = xpool.tile([64, 1024], dt.bfloat16, tag="rD")
        # C[i] = r[2i] + r[2i+1]: i 0..15 from rlo, 16..31 from rup
        nc.vector.tensor_tensor(
            _fview(rC[:], 0, [(32, 16), (1, 32)]),
            _fview(rlo, 0, [(64, 16), (1, 32)]),
            _fview(rlo, 32, [(64, 16), (1, 32)]), ALU.add,
        )
        nc.vector.tensor_tensor(
            _fview(rC[:], 512, [(32, 16), (1, 32)]),
            _fview(rup, 0, [(64, 16), (1, 32)]),
            _fview(rup, 32, [(64, 16), (1, 32)]), ALU.add,
        )
        # D[i] = r[2i-1] + r[2i+2]: i 1..14 lo, 15/16 cross, 17..30 up
        nc.vector.tensor_tensor(
            _fview(rD[:], 32, [(32, 14), (1, 32)]),
            _fview(rlo, 32, [(64, 14), (1, 32)]),
            _fview(rlo, 128, [(64, 14), (1, 32)]), ALU.add,
        )
        nc.vector.tensor_tensor(
            _fview(rD[:], 15 * 32, [(32, 1), (1, 32)]),
            _fview(rlo, 29 * 32, [(32, 1), (1, 32)]),
            _fview(rup, 0, [(32, 1), (1, 32)]), ALU.add,
        )
        nc.vector.tensor_tensor(
            _fview(rD[:], 16 * 32, [(32, 1), (1, 32)]),
            _fview(rlo, 31 * 32, [(32, 1), (1, 32)]),
            _fview(rup, 2 * 32, [(32, 1), (1, 32)]), ALU.add,
        )
        nc.vector.tensor_tensor(
            _fview(rD[:], 17 * 32, [(32, 14), (1, 32)]),
            _fview(rup, 32, [(64, 14), (1, 32)]),
            _fview(rup, 128, [(64, 14), (1, 32)]), ALU.add,
        )
        nc.vector.scalar_tensor_tensor(
            _fview(xc64, F2R0 + 32, [(32, 30), (1, 32)]),
            _fview(rC[:], 32, [(32, 30), (1, 32)]), 3.0,
            _fview(rD[:], 32, [(32, 30), (1, 32)]), ALU.mult, ALU.add,
        )
        oi0 = _fview(xc64, F2R0, [(32, 1), (1, 32)])
        nc.vector.tensor_scalar_mul(oi0, _fview(rC[:], 0, [(32, 1), (1, 32)]), 24.0 / 7.0)
        nc.vector.scalar_tensor_tensor(
            oi0, _fview(rlo, 2 * 32, [(32, 1), (1, 32)]), 8.0 / 7.0, oi0, ALU.mult, ALU.add
        )
        oi31 = _fview(xc64, F2R0 + 31 * 32, [(32, 1), (1, 32)])
        nc.vector.tensor_scalar_mul(oi31, _fview(rC[:], 31 * 32, [(32, 1), (1, 32)]), 24.0 / 7.0)
        nc.vector.scalar_tensor_tensor(
            oi31, _fview(rup, 29 * 32, [(32, 1), (1, 32)]), 8.0 / 7.0, oi31, ALU.mult, ALU.add
        )

        # ================= conv9 -> xbuf ======================================
        # k-outer: xcat ct0..2 are ready before ct3 (resize tail), so 24 of
        # the 32 matmuls can stream while the resize finishes.
        ps9 = [
            pspool.tile([128, 512], dt.float32, tag="ps", name=f"ps9_{m}_{n}")
            for m in range(4) for n in range(2)
        ]
        for k in range(4):
            for m in range(4):
                for n in range(2):
                    nc.tensor.matmul(
                        ps9[m * 2 + n][:, :],
                        w9sb[:, k * 512 + m * 128 : k * 512 + (m + 1) * 128],
                        xcat[:, k * 1024 + n * 512 : k * 1024 + (n + 1) * 512],
                        start=(k == 0),
                        stop=(k == 3),
                    )
        for m in range(4):
            for n in range(2):
                elu(
                    ps9[m * 2 + n], 0, 128,
                    xbuf[:, m * 1024 + n * 512 : m * 1024 + (n + 1) * 512],
                    pool_stt=True,
                )

        # xodd[p, j] = xbuf[p, j+1]  (for 4B-aligned odd-dx views); per-ctile
        # DMA shift-copies keep it off the DVE and start as soon as each
        # conv9 m-tile lands.
        for ct in range(4):
            nc.sync.dma_start(
                xodd[:, ct * 1024 : ct * 1024 + 1023],
                xbuf[:, ct * 1024 + 1 : ct * 1024 + 1024],
            )

        # S[pos] = sum_c x[c, pos] (rows 0-31 identical); s_corr = -S/2 in bf16
        ps_s = [
            pspool.tile([128, 512], dt.float32, tag="ps", name=f"ps_s{n}")
            for n in range(2)
        ]
        for n in range(2):
            for ct in range(4):
                nc.tensor.matmul(
                    ps_s[n][0:32, :],
                    ones[:, 0:32],
                    xbuf[:, ct * 1024 + n * 512 : ct * 1024 + (n + 1) * 512],
                    start=(ct == 0),
                    stop=(ct == 3),
                )
        # s_corr rows: hi = bf16(-S/2), lo = residual (-S/2 - hi); k=2 matmul
        # sums both, recovering ~fp32 accuracy from bf16 operands.
        s_tmp = xpool.tile([1, 2048], dt.bfloat16, tag="s_tmp")
        for n in range(2):
            nc.scalar.activation(
                s_tmp[0:1, n * 512 : (n + 1) * 512], ps_s[n][0:1, :], ACTF.Copy, scale=-0.5
            )
            nc.vector.scalar_tensor_tensor(
                s_tmp[0:1, 1024 + n * 512 : 1024 + (n + 1) * 512],
                ps_s[n][0:1, :], -0.5,
                s_tmp[0:1, n * 512 : (n + 1) * 512],
                ALU.mult, ALU.subtract,
            )
        # scorr rows 0/1: hi = bf16(-S/2), lo = residual; rows 2..127 zero so
        # the correction matmuls can be K=128 (same (128,32) PE tile mode and
        # same all-ones lhsT as the channel-sum matmuls -> no mode switches).
        nc.sync.dma_start(scorr[0:1, :], s_tmp[0:1, 0:1024])
        nc.sync.dma_start(scorr[1:2, :], s_tmp[0:1, 1024:2048])

        # ================= affinity ==========================================
        # Per bank-quad: 4 slots run on the 4 (128,32) PE column tiles
        # concurrently (interleaved issue), 6 chained K=128 matmuls each
        # (4 channel-sum + 2 corrections vs zero-padded scorr). One ACT exp
        # over the partition-strided rows {0,32,64,96} extracts the quad.
        xba = xbuf[:]
        atiles_by_group = {}

        def emit_max(g, ct):
            """One [128, G] max tile; ct==3 runs on Pool (DVE is the phase
            bottleneck, Pool is idle -> ~25% of the max work moves over)."""
            dy, dxs = GROUPS[g]
            ndx = len(dxs)
            G = ndx * NPOS
            odd = dxs[0] % 2 != 0
            ctb = ct * 1024
            ff = _fview(xba, ctb + 4, [(0, ndx), (32, CH), (1, CW)])
            if odd:
                ft = _fview(
                    xodd[:], ctb + 32 * dy + 4 + dxs[0] - 1,
                    [(2, ndx), (32, CH), (1, CW)],
                )
            else:
                ft = _fview(
                    xba, ctb + 32 * dy + 4 + dxs[0],
                    [(2, ndx), (32, CH), (1, CW)],
                )
            mtile = dpool.tile([128, G], dt.bfloat16, tag="d", name=f"m_{g}_{ct}")
            mv = _fview(mtile[:], 0, [(NPOS, ndx), (CW, CH), (1, CW)])
            nc.vector.tensor_tensor(mv, ft, ff, ALU.max)
            return mtile

        def ensure_group(g):
            if g in atiles_by_group:
                return
            atiles_by_group[g] = [emit_max(g, ct) for ct in range(4)]

        # First two groups ct-major: their ct-k max ops only need conv9's
        # m=k output tile, so they start while conv9 is still streaming.
        atiles_by_group[0] = [None] * 4
        atiles_by_group[1] = [None] * 4
        for ct in range(4):
            for g in (0, 1):
                atiles_by_group[g][ct] = emit_max(g, ct)

        for t in range(N_BANKSEQ):
            slots = list(range(4 * t, 4 * t + 4))
            for s in slots:
                ensure_group(_SLOTS[s][0])
            pst = pspool.tile([128, 512], dt.float32, tag="ps", name=f"pq_{t}")
            for k in range(6):
                for q, s in enumerate(slots):
                    g, c0, cs = _SLOTS[s]
                    dy, dxs = GROUPS[g]
                    if k < 4:
                        rhs = atiles_by_group[g][k][:, c0 : c0 + cs]
                    else:
                        dxi, h2 = (c0 // NPOS), (c0 % NPOS) // 336
                        off = 4 + 32 * 14 * h2
                        if k == 5:
                            off += 32 * dy + dxs[dxi]
                        rhs = _fview(scorr[:], off, [(32, 14), (1, CW)])
                    nc.tensor.matmul(
                        pst[32 * q : 32 * q + 32, 0:cs],
                        ones[:, 0:32],
                        rhs,
                        start=(k == 0),
                        stop=(k == 5),
                        tile_position=(0, 32 * q),
                    )
            # ACT cost scales with free size only -> exp the whole tile (every
            # row of col-tile q holds slot q's sum); the DMA, which does
            # support partition strides, picks rows {0,32,64,96}.
            affb = opool.tile([128, CHUNK], dt.float32, tag="affb")
            nc.scalar.activation(
                affb[:, :], pst[:, 0:CHUNK], ACTF.Exp, scale=-1.0 / 256.0
            )
            nc.sync.dma_start(
                outd[4 * t : 4 * t + 4, :], _pstride_view(affb[:], 4, CHUNK)
            )

    nc.compile()
    return nc


# ------------------------------ host wrapper ---------------------------------

_NC_CACHE = None
LAST_EXEC_NS = None
LAST_MEAN_EXEC_NS = None


def _get_nc():
    global _NC_CACHE
    if _NC_CACHE is None:
        _NC_CACHE = build_nc()
    return _NC_CACHE


def _prep_inputs(f2_in, f3_in, f4_in, w2, w3, w4, w9):
    """Shard/tile/cast on host. Returns per-core input maps."""

    def ctile(a, k):  # [C, S] -> [128, k*S] with c-tile t at cols [t*S, (t+1)*S)
        c, s = a.shape
        assert c == 128 * k
        return np.ascontiguousarray(
            a.reshape(k, 128, s).transpose(1, 0, 2).reshape(128, k * s)
        )

    w2t = ctile(np.asarray(w2, np.float32).T.astype(BF16), 4)      # [512,64]
    w3t = ctile(np.asarray(w3, np.float32).T.astype(BF16), 8)      # [1024,128]
    w4t = ctile(np.asarray(w4, np.float32).T.astype(BF16), 16)     # [2048,320]
    w9p = np.asarray(w9, np.float32)[:, XCAT_PERM].T               # [512 in, 512 out]
    w9p = w9p.copy()
    w9p[384:448, :] *= 1.0 / 64.0  # f2r rows: resize passes leave a 64x scale
    w9t = ctile(w9p.astype(BF16), 4)

    f2 = np.asarray(f2_in, np.float32).reshape(B, 512, 4096).astype(BF16)
    f3 = np.asarray(f3_in, np.float32).reshape(B, 1024, 1024).astype(BF16)
    f4 = np.asarray(f4_in, np.float32).reshape(B, 2048, 1024).astype(BF16)

    in_maps = []
    for b in range(B):
        in_maps.append(
            {
                "f2": ctile(f2[b], 4),
                "f3": ctile(f3[b], 8),
                "f4": ctile(f4[b], 16),
                "w2t": w2t,
                "w3t": w3t,
                "w4t": w4t,
                "w9t": w9t,
            }
        )
    return in_maps


def _install_trace_hooks():
    import types

    if "antenv.axon_hooks" not in sys.modules:
        mod = types.ModuleType("antenv.axon_hooks")
        _HOOK = [None]
        mod.set_axon_ntff_profile_hook = lambda h: _HOOK.__setitem__(0, h)
        mod.get_axon_ntff_profile_hook = lambda: _HOOK[0]
        sys.modules["antenv.axon_hooks"] = mod
        from trn_agent_boot.trn_boot import _ntff_profile_via_ctypes

        mod.set_axon_ntff_profile_hook(
            _ntff_profile_via_ctypes("/opt/axon/libaxon_pjrt.so")
        )
    import concourse.bass_utils as bass_utils

    bass_utils.upload_artifacts = lambda tmpdir: f"local:{tmpdir}"


def kernel(f2_in, f3_in, f4_in, w2, w3, w4, w9, _trace=False, _tmpdir=None):
    global LAST_EXEC_NS, LAST_MEAN_EXEC_NS
    from concourse.bass_utils import run_bass_kernel_spmd

    if _trace:
        _install_trace_hooks()

    nc = _get_nc()
    in_maps = _prep_inputs(f2_in, f3_in, f4_in, w2, w3, w4, w9)
    res = run_bass_kernel_spmd(
        nc, in_maps, list(range(N_CORES)), trace=_trace, tmpdir=_tmpdir
    )
    LAST_EXEC_NS = res.exec_time_ns
    LAST_MEAN_EXEC_NS = res.mean_exec_time_ns

    out = np.empty((B, 34, NPOS), np.float32)
    for b in range(B):
        flat = res.results[b]["aff"].reshape(-1)
        out[b] = flat[_FLAT_IDX]
    return out

